# revision 4
# baseline (speedup 1.0000x reference)
"""MaxUnpooling2D scatter-add kernel for Trainium2 (8 NeuronCores, batch-sharded).

Problem: inputs [8,128,128,64] f32, pooling_indices [8,128,128,64] i32 in
[0, 256*256*64). Output [8,256,256,64] f32 with out[b,y,x,c] += v where
y = idx//16384, x = (idx//64)%256 and channel c preserved (duplicates sum).

Strategy (per core = one batch):
  Since the destination channel equals the source channel, the scatter is
  independent per (b, c) plane: 16384 elements -> 65536 bins. The in-plane
  destination is idx >> 6. Each plane's 65536 bins live in one PSUM bank
  laid out [128 x 512] (po = dest>>9, f = dest&511). For each group of 128
  elements (one per SBUF partition: partition = h, group = (w, c) column),
  a selection matmul accumulates them into the plane's bank:
     lhsT[e, po] = onehot128(po_e)        (fp16)
     rhs [e, f ] = v_e * onehot512(f_e)   (fp16)
     bank += lhsT.T @ rhs                 (PSUM f32 accumulate)

  Measured engine rates (see work/probe_* and trace analyses): DVE
  tensor_scalar ~111ns fixed + 0.414ns/elem, Act Square+Exp one-hot
  ~650ns, Pool/GPSIMD tensor ops ~3us (unusable), indirect DMA broken
  for any real indirection. The binding resource is DVE's 8192 rhs
  builds (~2.67ms); balance the rest so no engine exceeds ~3.3ms:
    rhs:  always DVE tensor_scalar (is_equal x value dual op)
    lhsT: 62.5% Act (Square+Exp delta), 37.5% DVE (is_equal)
    PSUM drains: alternated DVE tensor_copy / Act Copy.
  Channels are processed in 8 waves of 8 (8 PSUM banks), drained to SBUF
  interleaved by channel and written to HBM as [.., 8-channel] runs.

Duplicate indices are exact: accumulation happens in f32 PSUM; the only
approximation is the fp16 rounding of each input value (~2e-4 relative).
"""
import sys
import numpy as np

for p in ("/opt/trn_rl_repo",):
    if p not in sys.path:
        sys.path.insert(0, p)

from contextlib import ExitStack
import concourse.bass as bass
import concourse.bacc as bacc
import concourse.tile as tile
import concourse.mybir as mybir
from concourse.bass_utils import run_bass_kernel_spmd

F32, F16, F8, I32 = mybir.dt.float32, mybir.dt.float16, mybir.dt.float8e4, mybir.dt.int32
AF = mybir.ActivationFunctionType
ALU = mybir.AluOpType

B, H, W, C = 8, 128, 128, 64
OH, OW = 256, 256
NW = 128  # w-groups per (h-partition, c)

# v3: rhs = PURE onehot512(f) in fp8 (single-op is_equal); v folded into lhsT.
# lhsT: every LHST_ACT_EVERY-th column built on Act (Square+Exp+Copy*scale),
# rest on DVE (dual-op is_equal*v). rhs: every RHS_POOL_EVERY-th on Pool.
LHST_ACT_EVERY = 4
RHS_POOL_EVERY = 8


def build(n_waves=8):
    nc = bacc.Bacc()
    x = nc.declare_dram_parameter("x", [128, 8192], F32, isOutput=False)
    idx = nc.declare_dram_parameter("idx", [128, 8192], I32, isOutput=False)
    out = nc.declare_dram_parameter("out", [65536, 64], F32, isOutput=True)
    out3 = out[:].rearrange("(po f) c -> po f c", f=512)

    with tile.TileContext(nc) as tc, ExitStack() as ctx:
        cpool = ctx.enter_context(tc.tile_pool(name="const", bufs=1))
        dpool = ctx.enter_context(tc.tile_pool(name="decoded", bufs=1))

        iota128_i = cpool.tile([128, 128], I32)
        nc.gpsimd.iota(iota128_i[:], [[1, 128]], base=0, channel_multiplier=0)
        iota128 = cpool.tile([128, 128], F16)
        nc.vector.tensor_copy(iota128[:], iota128_i[:])
        iota128fneg = cpool.tile([128, 128], F32)
        nc.vector.tensor_scalar(iota128fneg[:], iota128_i[:], -1, None, ALU.mult)
        iota512_i = cpool.tile([128, 512], I32)
        nc.gpsimd.iota(iota512_i[:], [[1, 512]], base=0, channel_multiplier=0)
        iota512 = cpool.tile([128, 512], F16)
        nc.vector.tensor_copy(iota512[:], iota512_i[:])

        with tc.tile_pool(name="inputs", bufs=1) as ipool:
            it = ipool.tile([128, 8192], I32)
            nc.sync.dma_start(it[:], idx[:])
            dest = ipool.tile([128, 8192], I32)
            nc.vector.tensor_scalar(dest[:], it[:], 6, None, ALU.logical_shift_right)
            tmp_i = ipool.tile([128, 8192], I32)
            nc.vector.tensor_scalar(tmp_i[:], dest[:], 9, None, ALU.logical_shift_right)
            po32 = dpool.tile([128, 8192], F32)
            nc.vector.tensor_copy(po32[:], tmp_i[:])
            nc.vector.tensor_scalar(tmp_i[:], dest[:], 511, None, ALU.bitwise_and)
            f32t = dpool.tile([128, 8192], F32)
            nc.vector.tensor_copy(f32t[:], tmp_i[:])
        xt = dpool.tile([128, 8192], F32)
        nc.sync.dma_start(xt[:], x[:])

        gpool = ctx.enter_context(tc.tile_pool(name="grp", bufs=16))
        drpool = ctx.enter_context(tc.tile_pool(name="drain", bufs=2))
        ppool = ctx.enter_context(tc.tile_pool(name="psum", bufs=1, space="PSUM"))
        gidx = 0
        for wave in range(n_waves):
            banks = [
                ppool.tile([128, 512], F32, tag=f"bank{c}", name=f"bank_w{wave}_{c}")
                for c in range(8)
            ]
            for w in range(NW):
                for ci in range(8):
                    c = wave * 8 + ci
                    col = w * 64 + c
                    gidx += 1
                    lhsT = gpool.tile([128, 128], F16, tag="lhsT")
                    rhs = gpool.tile([128, 512], F8, tag="rhs")
                    # rhs = onehot512(f), pure, fp8 (exact 1.0)
                    if gidx % RHS_POOL_EVERY == 0:
                        nc.gpsimd.tensor_scalar(
                            rhs[:], iota512[:], f32t[:, col:col + 1],
                            None, ALU.is_equal,
                        )
                    else:
                        nc.vector.tensor_scalar(
                            rhs[:], iota512[:], f32t[:, col:col + 1],
                            None, ALU.is_equal,
                        )
                    # lhsT = v * onehot128(po)
                    if gidx % LHST_ACT_EVERY == 0:
                        t2 = gpool.tile([128, 128], F32, tag="t2")
                        nc.scalar.activation(
                            t2[:], iota128fneg[:], AF.Square,
                            bias=po32[:, col:col + 1], scale=1.0,
                        )
                        oh = gpool.tile([128, 128], F32, tag="oh")
                        nc.scalar.activation(oh[:], t2[:], AF.Exp, scale=-16.0)
                        nc.scalar.activation(
                            lhsT[:], oh[:], AF.Copy, scale=xt[:, col:col + 1],
                        )
                    else:
                        nc.vector.tensor_scalar(
                            lhsT[:], iota128[:], po32[:, col:col + 1],
                            xt[:, col:col + 1], ALU.is_equal, ALU.mult,
                        )
                    nc.tensor.matmul(
                        out=banks[ci][:], lhsT=lhsT[:], rhs=rhs[:],
                        start=(w == 0), stop=(w == NW - 1),
                    )
            S = drpool.tile([128, 4096], F32, tag="S")
            s3 = S[:].rearrange("p (f c) -> p f c", c=8)
            for ci in range(8):
                if ci % 2 == 0:
                    nc.vector.tensor_copy(s3[:, :, ci], banks[ci][:])
                else:
                    nc.scalar.activation(
                        s3[:, :, ci], banks[ci][:], AF.Copy, scale=1.0
                    )
            s3v = S[:].rearrange("p (f c) -> p f c", c=8)
            nc.sync.dma_start(out3[0:64, :, wave * 8: wave * 8 + 8], s3v[0:64])
            nc.sync.dma_start(out3[64:128, :, wave * 8: wave * 8 + 8], s3v[64:128])
    nc.compile()
    return nc


_NC_CACHE = {}


def _get_nc():
    if "nc" not in _NC_CACHE:
        _NC_CACHE["nc"] = build(8)
    return _NC_CACHE["nc"]


def run(in_maps, trace=False, tmpdir=None):
    nc = _get_nc()
    return run_bass_kernel_spmd(
        nc, in_maps, core_ids=list(range(8)), trace=trace, tmpdir=tmpdir
    )


def kernel(inputs: np.ndarray, pooling_indices: np.ndarray) -> np.ndarray:
    inputs = np.asarray(inputs, dtype=np.float32)
    pooling_indices = np.asarray(pooling_indices)
    assert inputs.shape == (B, H, W, C) and pooling_indices.shape == (B, H, W, C)
    in_maps = [
        {
            "x": np.ascontiguousarray(inputs[b].reshape(H, W * C), dtype=np.float32),
            "idx": np.ascontiguousarray(
                pooling_indices[b].reshape(H, W * C).astype(np.int32)
            ),
        }
        for b in range(B)
    ]
    res = run(in_maps)
    out = np.stack([res.results[b]["out"] for b in range(B)], axis=0)
    return out.reshape(B, OH, OW, C).astype(np.float32)



# revision 5
# speedup vs baseline: 3.3531x; 3.3531x over previous
"""MaxUnpooling2D scatter-add kernel for Trainium2 (8 NeuronCores, batch-sharded).

Problem: inputs [8,128,128,64] f32, pooling_indices [8,128,128,64] i32 in
[0, 256*256*64). Output [8,256,256,64] f32 with out[b,y,x,c] += v where
y = idx//16384, x = (idx//64)%256 and channel c preserved (duplicates sum).

Strategy (per core = one batch):
  Since the destination channel equals the source channel, the scatter is
  independent per (b, c) plane: 16384 elements -> 65536 bins. The in-plane
  destination is idx >> 6. Each plane's 65536 bins live in one PSUM bank
  laid out [128 x 512] (po = dest>>9, f = dest&511). For each group of 128
  elements (one per SBUF partition: partition = h, group = (w, c) column),
  a selection matmul accumulates them into the plane's bank:
     lhsT[e, po] = onehot128(po_e)        (fp16)
     rhs [e, f ] = v_e * onehot512(f_e)   (fp16)
     bank += lhsT.T @ rhs                 (PSUM f32 accumulate)

  Measured engine rates (see work/probe_* and trace analyses): DVE
  tensor_scalar ~111ns fixed + 0.414ns/elem, Act Square+Exp one-hot
  ~650ns, Pool/GPSIMD tensor ops ~3us (unusable), indirect DMA broken
  for any real indirection. The binding resource is DVE's 8192 rhs
  builds (~2.67ms); balance the rest so no engine exceeds ~3.3ms:
    rhs:  always DVE tensor_scalar (is_equal x value dual op)
    lhsT: 62.5% Act (Square+Exp delta), 37.5% DVE (is_equal)
    PSUM drains: alternated DVE tensor_copy / Act Copy.
  Channels are processed in 8 waves of 8 (8 PSUM banks), drained to SBUF
  interleaved by channel and written to HBM as [.., 8-channel] runs.

Duplicate indices are exact: accumulation happens in f32 PSUM; the only
approximation is the fp16 rounding of each input value (~2e-4 relative).
"""
import sys
import numpy as np

for p in ("/opt/trn_rl_repo",):
    if p not in sys.path:
        sys.path.insert(0, p)

from contextlib import ExitStack
import concourse.bass as bass
import concourse.bacc as bacc
import concourse.tile as tile
import concourse.mybir as mybir
from concourse.bass_utils import run_bass_kernel_spmd

F32, F16, I32 = mybir.dt.float32, mybir.dt.float16, mybir.dt.int32
AF = mybir.ActivationFunctionType
ALU = mybir.AluOpType

B, H, W, C = 8, 128, 128, 64
OH, OW = 256, 256
NW = 128  # w-groups per (h-partition, c)

# lhsT engine rotation out of 40 slots: Act 16, DVE 24
LHST_POOL, LHST_ACT = 0, 16
# rhs: Pool disabled (gpsimd tensor ops are ~3us each)
RHS_POOL_EVERY = 1 << 30


def build(n_waves=8):
    nc = bacc.Bacc()
    x = nc.declare_dram_parameter("x", [128, 8192], F32, isOutput=False)
    idx = nc.declare_dram_parameter("idx", [128, 8192], I32, isOutput=False)
    out = nc.declare_dram_parameter("out", [65536, 64], F32, isOutput=True)
    out3 = out[:].rearrange("(po f) c -> po f c", f=512)

    with tile.TileContext(nc) as tc, ExitStack() as ctx:
        cpool = ctx.enter_context(tc.tile_pool(name="const", bufs=1))
        dpool = ctx.enter_context(tc.tile_pool(name="decoded", bufs=1))

        iota128_i = cpool.tile([128, 128], I32)
        nc.gpsimd.iota(iota128_i[:], [[1, 128]], base=0, channel_multiplier=0)
        iota128 = cpool.tile([128, 128], F16)
        nc.vector.tensor_copy(iota128[:], iota128_i[:])
        iota128fneg = cpool.tile([128, 128], F32)
        nc.vector.tensor_scalar(iota128fneg[:], iota128_i[:], -1, None, ALU.mult)
        iota512_i = cpool.tile([128, 512], I32)
        nc.gpsimd.iota(iota512_i[:], [[1, 512]], base=0, channel_multiplier=0)
        iota512 = cpool.tile([128, 512], F16)
        nc.vector.tensor_copy(iota512[:], iota512_i[:])

        with tc.tile_pool(name="inputs", bufs=1) as ipool:
            it = ipool.tile([128, 8192], I32)
            nc.sync.dma_start(it[:], idx[:])
            dest = ipool.tile([128, 8192], I32)
            nc.vector.tensor_scalar(dest[:], it[:], 6, None, ALU.logical_shift_right)
            tmp_i = ipool.tile([128, 8192], I32)
            nc.vector.tensor_scalar(tmp_i[:], dest[:], 9, None, ALU.logical_shift_right)
            po32 = dpool.tile([128, 8192], F32)
            nc.vector.tensor_copy(po32[:], tmp_i[:])
            nc.vector.tensor_scalar(tmp_i[:], dest[:], 511, None, ALU.bitwise_and)
            f32t = dpool.tile([128, 8192], F32)
            nc.vector.tensor_copy(f32t[:], tmp_i[:])
        xt = dpool.tile([128, 8192], F32)
        nc.sync.dma_start(xt[:], x[:])

        gpool = ctx.enter_context(tc.tile_pool(name="grp", bufs=16))
        drpool = ctx.enter_context(tc.tile_pool(name="drain", bufs=2))
        ppool = ctx.enter_context(tc.tile_pool(name="psum", bufs=1, space="PSUM"))
        gidx = 0
        for wave in range(n_waves):
            banks = [
                ppool.tile([128, 512], F32, tag=f"bank{c}", name=f"bank_w{wave}_{c}")
                for c in range(8)
            ]
            for w in range(NW):
                for ci in range(8):
                    c = wave * 8 + ci
                    col = w * 64 + c
                    gidx += 1
                    lhsT = gpool.tile([128, 128], F16, tag="lhsT")
                    rhs = gpool.tile([128, 512], F16, tag="rhs")
                    # rhs = v * onehot512(f)
                    if gidx % RHS_POOL_EVERY == 0:
                        nc.gpsimd.tensor_scalar(
                            rhs[:], iota512[:], f32t[:, col:col + 1],
                            xt[:, col:col + 1], ALU.is_equal, ALU.mult,
                        )
                    else:
                        nc.vector.tensor_scalar(
                            rhs[:], iota512[:], f32t[:, col:col + 1],
                            xt[:, col:col + 1], ALU.is_equal, ALU.mult,
                        )
                    # lhsT = onehot128(po)
                    slot = gidx % 40
                    if slot < LHST_POOL:
                        nc.gpsimd.tensor_scalar(
                            lhsT[:], iota128[:], po32[:, col:col + 1],
                            None, ALU.is_equal,
                        )
                    elif slot < LHST_ACT:
                        # exp(-16*(iota-po)^2) is an exact one-hot for ints
                        t2 = gpool.tile([128, 128], F32, tag="t2")
                        nc.scalar.activation(
                            t2[:], iota128fneg[:], AF.Square,
                            bias=po32[:, col:col + 1], scale=1.0,
                        )
                        nc.scalar.activation(lhsT[:], t2[:], AF.Exp, scale=-16.0)
                    else:
                        nc.vector.tensor_scalar(
                            lhsT[:], iota128[:], po32[:, col:col + 1],
                            None, ALU.is_equal,
                        )
                    nc.tensor.matmul(
                        out=banks[ci][:], lhsT=lhsT[:], rhs=rhs[:],
                        start=(w == 0), stop=(w == NW - 1),
                    )
            S = drpool.tile([128, 4096], F32, tag="S")
            s3 = S[:].rearrange("p (f c) -> p f c", c=8)
            for ci in range(8):
                if ci % 2 == 0:
                    nc.vector.tensor_copy(s3[:, :, ci], banks[ci][:])
                else:
                    nc.scalar.activation(
                        s3[:, :, ci], banks[ci][:], AF.Copy, scale=1.0
                    )
            s3v = S[:].rearrange("p (f c) -> p f c", c=8)
            nc.sync.dma_start(out3[0:64, :, wave * 8: wave * 8 + 8], s3v[0:64])
            nc.sync.dma_start(out3[64:128, :, wave * 8: wave * 8 + 8], s3v[64:128])
    nc.compile()
    return nc


_NC_CACHE = {}


def _get_nc():
    if "nc" not in _NC_CACHE:
        _NC_CACHE["nc"] = build(8)
    return _NC_CACHE["nc"]


def run(in_maps, trace=False, tmpdir=None):
    nc = _get_nc()
    return run_bass_kernel_spmd(
        nc, in_maps, core_ids=list(range(8)), trace=trace, tmpdir=tmpdir
    )


def kernel(inputs: np.ndarray, pooling_indices: np.ndarray) -> np.ndarray:
    inputs = np.asarray(inputs, dtype=np.float32)
    pooling_indices = np.asarray(pooling_indices)
    assert inputs.shape == (B, H, W, C) and pooling_indices.shape == (B, H, W, C)
    in_maps = [
        {
            "x": np.ascontiguousarray(inputs[b].reshape(H, W * C), dtype=np.float32),
            "idx": np.ascontiguousarray(
                pooling_indices[b].reshape(H, W * C).astype(np.int32)
            ),
        }
        for b in range(B)
    ]
    res = run(in_maps)
    out = np.stack([res.results[b]["out"] for b in range(B)], axis=0)
    return out.reshape(B, OH, OW, C).astype(np.float32)



# revision 6
# speedup vs baseline: 3.6569x; 1.0906x over previous
"""MaxUnpooling2D scatter-add kernel for Trainium2 (8 NeuronCores, batch-sharded).

Problem: inputs [8,128,128,64] f32, pooling_indices [8,128,128,64] i32 in
[0, 256*256*64). Output [8,256,256,64] f32 with out[b,y,x,c] += v where
y = idx//16384, x = (idx//64)%256 and channel c preserved (duplicates sum).

Strategy (per core = one batch):
  Since the destination channel equals the source channel, the scatter is
  independent per (b, c) plane: 16384 elements -> 65536 bins. The in-plane
  destination is idx >> 6. Each plane's 65536 bins live in one PSUM bank
  laid out [128 x 512] (po = dest>>9, f = dest&511). For each group of 128
  elements (one per SBUF partition: partition = h, group = (w, c) column),
  a selection matmul accumulates them into the plane's bank:
     lhsT[e, po] = onehot128(po_e)        (fp16)
     rhs [e, f ] = v_e * onehot512(f_e)   (fp16)
     bank += lhsT.T @ rhs                 (PSUM f32 accumulate)

  Measured engine rates (see work/probe_* and trace analyses): DVE
  tensor_scalar ~111ns fixed + 0.414ns/elem, Act Square+Exp one-hot
  ~650ns, Pool/GPSIMD tensor ops ~3us (unusable), indirect DMA broken
  for any real indirection. The binding resource is DVE's 8192 rhs
  builds (~2.67ms); balance the rest so no engine exceeds ~3.3ms:
    rhs:  always DVE tensor_scalar (is_equal x value dual op)
    lhsT: 62.5% Act (Square+Exp delta), 37.5% DVE (is_equal)
    PSUM drains: alternated DVE tensor_copy / Act Copy.
  Channels are processed in 8 waves of 8 (8 PSUM banks), drained to SBUF
  interleaved by channel and written to HBM as [.., 8-channel] runs.

Duplicate indices are exact: accumulation happens in f32 PSUM; the only
approximation is the fp16 rounding of each input value (~2e-4 relative).
"""
import sys
import numpy as np

for p in ("/opt/trn_rl_repo",):
    if p not in sys.path:
        sys.path.insert(0, p)

from contextlib import ExitStack
import concourse.bass as bass
import concourse.bacc as bacc
import concourse.tile as tile
import concourse.mybir as mybir
from concourse.bass_utils import run_bass_kernel_spmd

F32, F16, I32 = mybir.dt.float32, mybir.dt.float16, mybir.dt.int32
AF = mybir.ActivationFunctionType
ALU = mybir.AluOpType

B, H, W, C = 8, 128, 128, 64
OH, OW = 256, 256
NW = 128  # w-groups per (h-partition, c)

# lhsT engine rotation out of 40 slots: Act 25, DVE 15
LHST_POOL, LHST_ACT = 0, 25
# rhs: Pool disabled (gpsimd tensor ops are ~3us each)
RHS_POOL_EVERY = 1 << 30


def build(n_waves=8):
    nc = bacc.Bacc()
    x = nc.declare_dram_parameter("x", [128, 8192], F32, isOutput=False)
    idx = nc.declare_dram_parameter("idx", [128, 8192], I32, isOutput=False)
    out = nc.declare_dram_parameter("out", [65536, 64], F32, isOutput=True)
    out3 = out[:].rearrange("(po f) c -> po f c", f=512)

    with tile.TileContext(nc) as tc, ExitStack() as ctx:
        cpool = ctx.enter_context(tc.tile_pool(name="const", bufs=1))
        dpool = ctx.enter_context(tc.tile_pool(name="decoded", bufs=1))

        iota128_i = cpool.tile([128, 128], I32)
        nc.gpsimd.iota(iota128_i[:], [[1, 128]], base=0, channel_multiplier=0)
        iota128 = cpool.tile([128, 128], F16)
        nc.vector.tensor_copy(iota128[:], iota128_i[:])
        iota128fneg = cpool.tile([128, 128], F32)
        nc.vector.tensor_scalar(iota128fneg[:], iota128_i[:], -1, None, ALU.mult)
        iota512_i = cpool.tile([128, 512], I32)
        nc.gpsimd.iota(iota512_i[:], [[1, 512]], base=0, channel_multiplier=0)
        iota512 = cpool.tile([128, 512], F16)
        nc.vector.tensor_copy(iota512[:], iota512_i[:])

        with tc.tile_pool(name="inputs", bufs=1) as ipool:
            it = ipool.tile([128, 8192], I32)
            nc.sync.dma_start(it[:], idx[:])
            dest = ipool.tile([128, 8192], I32)
            nc.vector.tensor_scalar(dest[:], it[:], 6, None, ALU.logical_shift_right)
            tmp_i = ipool.tile([128, 8192], I32)
            nc.vector.tensor_scalar(tmp_i[:], dest[:], 9, None, ALU.logical_shift_right)
            po32 = dpool.tile([128, 8192], F32)
            nc.vector.tensor_copy(po32[:], tmp_i[:])
            nc.vector.tensor_scalar(tmp_i[:], dest[:], 511, None, ALU.bitwise_and)
            f32t = dpool.tile([128, 8192], F32)
            nc.vector.tensor_copy(f32t[:], tmp_i[:])
        xt = dpool.tile([128, 8192], F32)
        nc.sync.dma_start(xt[:], x[:])

        gpool = ctx.enter_context(tc.tile_pool(name="grp", bufs=16))
        drpool = ctx.enter_context(tc.tile_pool(name="drain", bufs=2))
        ppool = ctx.enter_context(tc.tile_pool(name="psum", bufs=1, space="PSUM"))
        gidx = 0
        for wave in range(n_waves):
            banks = [
                ppool.tile([128, 512], F32, tag=f"bank{c}", name=f"bank_w{wave}_{c}")
                for c in range(8)
            ]
            for w in range(NW):
                for ci in range(8):
                    c = wave * 8 + ci
                    col = w * 64 + c
                    gidx += 1
                    lhsT = gpool.tile([128, 128], F16, tag="lhsT")
                    rhs = gpool.tile([128, 512], F16, tag="rhs")
                    # rhs = v * onehot512(f)
                    if gidx % RHS_POOL_EVERY == 0:
                        nc.gpsimd.tensor_scalar(
                            rhs[:], iota512[:], f32t[:, col:col + 1],
                            xt[:, col:col + 1], ALU.is_equal, ALU.mult,
                        )
                    else:
                        nc.vector.tensor_scalar(
                            rhs[:], iota512[:], f32t[:, col:col + 1],
                            xt[:, col:col + 1], ALU.is_equal, ALU.mult,
                        )
                    # lhsT = onehot128(po)
                    slot = gidx % 40
                    if slot < LHST_POOL:
                        nc.gpsimd.tensor_scalar(
                            lhsT[:], iota128[:], po32[:, col:col + 1],
                            None, ALU.is_equal,
                        )
                    elif slot < LHST_ACT:
                        # exp(-16*(iota-po)^2) is an exact one-hot for ints
                        t2 = gpool.tile([128, 128], F32, tag="t2")
                        nc.scalar.activation(
                            t2[:], iota128fneg[:], AF.Square,
                            bias=po32[:, col:col + 1], scale=1.0,
                        )
                        nc.scalar.activation(lhsT[:], t2[:], AF.Exp, scale=-16.0)
                    else:
                        nc.vector.tensor_scalar(
                            lhsT[:], iota128[:], po32[:, col:col + 1],
                            None, ALU.is_equal,
                        )
                    nc.tensor.matmul(
                        out=banks[ci][:], lhsT=lhsT[:], rhs=rhs[:],
                        start=(w == 0), stop=(w == NW - 1),
                    )
            S = drpool.tile([128, 4096], F32, tag="S")
            s3 = S[:].rearrange("p (f c) -> p f c", c=8)
            for ci in range(8):
                if ci % 2 == 0:
                    nc.vector.tensor_copy(s3[:, :, ci], banks[ci][:])
                else:
                    nc.scalar.activation(
                        s3[:, :, ci], banks[ci][:], AF.Copy, scale=1.0
                    )
            s3v = S[:].rearrange("p (f c) -> p f c", c=8)
            nc.sync.dma_start(out3[0:64, :, wave * 8: wave * 8 + 8], s3v[0:64])
            nc.sync.dma_start(out3[64:128, :, wave * 8: wave * 8 + 8], s3v[64:128])
    nc.compile()
    return nc


_NC_CACHE = {}


def _get_nc():
    if "nc" not in _NC_CACHE:
        _NC_CACHE["nc"] = build(8)
    return _NC_CACHE["nc"]


def run(in_maps, trace=False, tmpdir=None):
    nc = _get_nc()
    return run_bass_kernel_spmd(
        nc, in_maps, core_ids=list(range(8)), trace=trace, tmpdir=tmpdir
    )


def kernel(inputs: np.ndarray, pooling_indices: np.ndarray) -> np.ndarray:
    inputs = np.asarray(inputs, dtype=np.float32)
    pooling_indices = np.asarray(pooling_indices)
    assert inputs.shape == (B, H, W, C) and pooling_indices.shape == (B, H, W, C)
    in_maps = [
        {
            "x": np.ascontiguousarray(inputs[b].reshape(H, W * C), dtype=np.float32),
            "idx": np.ascontiguousarray(
                pooling_indices[b].reshape(H, W * C).astype(np.int32)
            ),
        }
        for b in range(B)
    ]
    res = run(in_maps)
    out = np.stack([res.results[b]["out"] for b in range(B)], axis=0)
    return out.reshape(B, OH, OW, C).astype(np.float32)

